# revision 22
# baseline (speedup 1.0000x reference)
"""Multi-head attention (B=4, N=2048, C=1024, H=16, D=64) on 8 TRN2 cores.

Sharding: core c -> batch b = c%4, head-group g = c//4 (local heads 0..7 are
global heads 8g..8g+7).  Each core computes its head group's contribution to
the output projection for its batch; host sums core b + core b+4 and adds
const_row = qkv_b[2048:] @ proj_w + proj_b (V-bias folds exactly through the
row-normalized attention: attn @ (1*bv^T) = 1*bv^T).

All matmul operands bf16 (fp32 PSUM accumulation).  Structure (v3):
- Input DMAs: x as 8 full 4KB-line rows spread over 4 queues; w full rows.
- Prefix: V (all tokens), K (pr0), Q (qb0,pr0) -- ~36us, DMA-overlapped.
- Steady state: 128 steps, PR-OUTER order (pr, qb, tg) so the remaining
  K (pr1-3) / Q projection groups trickle at a uniform ~1.7 matmuls/step
  into per-step PE slack (deadline-driven queue; qacc bank bufs=1).
  ACT-paced at 2x1147ns/step.
- Score PSUM staging grouped BY KEY TILE (stageA = t0 x [h0|h1]) so the
  tile_position row-half pair (0,0)/(64,0) shares one exp dependency,
  stays adjacent in the PE queue, and co-runs on the PE array.
- PSUM budget (8 banks): stage 2x2 + oaug 2x1 + qacc 1 + pj 1.
- Softmax denominator rides as a 65th ones-column in V (attn@1 = rowsum).
  Normalization: denom row -> SBUF -> DRAM bounce reshaped [64,8] for a
  wide DVE reciprocal -> broadcast read -> deferred multiply; the two
  heads' bounce chains use different DMA queues.
- Projections: with pr-outer order O_qb(qb) completes only after its pr3
  normalize, so proj blocks run 1/step over the last ~24 steps and qb3's
  in a pipelined tail (pj/qacc bank alternation, pr0-2 partials emitted
  early, a few spaced dummies keep HAM warm through the bounce latency).
- gpsimd partition_broadcast / reciprocal_approx_fast are numerically
  broken on this hardware - do not reintroduce them.
"""

import sys

sys.path.insert(0, "/opt/trn_rl_repo")

from contextlib import ExitStack

import ml_dtypes
import numpy as np

from concourse import bacc, mybir, tile
from concourse.bass_utils import run_bass_kernel_spmd

F32 = mybir.dt.float32
BF16 = mybir.dt.bfloat16
EXP = mybir.ActivationFunctionType.Exp
ADD = mybir.AluOpType.add
MULT = mybir.AluOpType.mult

B, N, C, H, D = 4, 2048, 1024, 16, 64
SCALE = 0.125
TB = 512


def _bf16(a: np.ndarray) -> np.ndarray:
    return np.ascontiguousarray(a, dtype=np.float32).astype(ml_dtypes.bfloat16)


class Trickle:
    """Deadline-driven queue of single-matmul closures fed into PE slack."""

    def __init__(self):
        self.items = []

    def add_group(self, deadline, closures):
        for c in closures:
            self.items.append((deadline, c))

    def emit(self, step, quota):
        n = 0
        while self.items and (self.items[0][0] <= step + 1 or n < quota):
            self.items.pop(0)[1]()
            n += 1

    def drain(self):
        while self.items:
            self.items.pop(0)[1]()


def _build():
    nc = bacc.Bacc("TRN2", target_bir_lowering=False, debug=False)
    xT16 = nc.dram_tensor("xT16", [1024, 2048], BF16, kind="ExternalInput").ap()
    wcat = nc.dram_tensor("wcat", [1024, 1536], BF16, kind="ExternalInput").ap()
    qb = nc.dram_tensor("qb", [128, 4], F32, kind="ExternalInput").ap()
    kb = nc.dram_tensor("kb", [128, 4], F32, kind="ExternalInput").ap()
    pw = nc.dram_tensor("pw", [512, 1024], BF16, kind="ExternalInput").ap()
    out = nc.dram_tensor("out", [2048, 1024], F32, kind="ExternalOutput").ap()
    scratch = nc.dram_tensor("scratch", [32, 512], F32).ap()
    scratch2 = nc.dram_tensor("scratch2", [32, 512], F32).ap()

    with tile.TileContext(nc) as tc, ExitStack() as ctx:
        sb = ctx.enter_context(tc.tile_pool(name="sb", bufs=1))
        ps = ctx.enter_context(tc.tile_pool(name="ps", bufs=1, space="PSUM"))

        w_sb = sb.tile([128, 8, 1536], BF16, tag="w")
        pw_sb = sb.tile([128, 4, 1024], BF16, tag="pw")
        Q_T = sb.tile([128, 4, 2048], BF16, tag="qt")
        K_T = sb.tile([128, 4, 2048], BF16, tag="kt")
        V_sb = sb.tile([128, 16, 8, 65], BF16, tag="v")
        x_sb = sb.tile([128, 8, 2048], BF16, tag="x")
        qb_sb = sb.tile([128, 4], F32, tag="qb")
        kb_sb = sb.tile([128, 4], F32, tag="kb")
        zc = sb.tile([128, 8, 1], F32, tag="zc")
        onec = sb.tile([128, 1], F32, tag="onec")
        warm = sb.tile([128, 4], F32, tag="warm")

        # ---- initial DMAs over the three DMA-capable queues, ordered by
        # first use: x token-halves in two waves (the prefix only needs
        # tokens 0-1023 early), w rows spread 4/2/2 so the V columns land
        # by ~12us.
        # prefix-critical 6MB (x + w KV-columns) balanced 2MB/engine and
        # interleaved so K/V chunk compute pipelines with arrivals; the w
        # Q-columns, second x wave, biases and pw follow.
        for j in range(0, 8, 2):
            nc.sync.dma_start(x_sb[:, j, 0:1024],
                              xT16[j * 128:(j + 1) * 128, 0:1024])
            nc.gpsimd.dma_start(x_sb[:, j + 1, 0:1024],
                                xT16[(j + 1) * 128:(j + 2) * 128, 0:1024])
            if j < 8:
                nc.scalar.dma_start(w_sb[:, j // 2, 512:1536],
                                    wcat[(j // 2) * 128:
                                         (j // 2 + 1) * 128, 512:1536])
        nc.sync.dma_start(w_sb[:, 4, 512:1536], wcat[4 * 128:5 * 128,
                                                     512:1536])
        nc.sync.dma_start(w_sb[:, 6, 512:1536], wcat[6 * 128:7 * 128,
                                                     512:1536])
        nc.gpsimd.dma_start(w_sb[:, 5, 512:1536], wcat[5 * 128:6 * 128,
                                                       512:1536])
        nc.gpsimd.dma_start(w_sb[:, 7, 512:1536], wcat[7 * 128:8 * 128,
                                                       512:1536])
        nc.scalar.dma_start(kb_sb[:], kb[:])
        nc.scalar.dma_start(qb_sb[:], qb[:])
        for j in range(4):  # Q-columns, first needed ~step 0-16
            nc.sync.dma_start(w_sb[:, j, 0:512],
                              wcat[j * 128:(j + 1) * 128, 0:512])
            nc.gpsimd.dma_start(w_sb[:, j + 4, 0:512],
                                wcat[(j + 4) * 128:(j + 5) * 128, 0:512])
        for j in range(0, 8, 2):
            nc.sync.dma_start(x_sb[:, j, 1024:2048],
                              xT16[j * 128:(j + 1) * 128, 1024:2048])
            nc.gpsimd.dma_start(x_sb[:, j + 1, 1024:2048],
                                xT16[(j + 1) * 128:(j + 2) * 128, 1024:2048])
        for pr in range(4):
            nc.scalar.dma_start(pw_sb[:, pr, :],
                                pw[pr * 128:(pr + 1) * 128, :])

        def x_tok(j, lo, n):
            return x_sb[:, j, lo:lo + n]

        nc.vector.memset(zc[:], 0.0)
        nc.vector.memset(onec[:], 1.0)
        # preload the exp table set while the prefix runs
        nc.scalar.activation(warm[0:1, 0:1], onec[0:1, 0:1], EXP,
                             bias=0.0, scale=1.0)
        for t in range(16):
            nc.vector.tensor_scalar(out=V_sb[:, t, :, 64:65], in0=zc[:],
                                    scalar1=onec[:], scalar2=None, op0=ADD)

        # ---- projection-group emitters (shared by prefix and trickle)
        def k_group(nb, pr, tag="qacc"):
            def mk(j):
                def mm():
                    if j == 0:
                        k_group.acc = ps.tile([128, TB], F32, tag=tag,
                                              bufs=1)
                    nc.tensor.matmul(
                        k_group.acc[:],
                        w_sb[:, j, 512 + pr * 128:512 + (pr + 1) * 128],
                        x_tok(j, nb * TB, TB), start=(j == 0), stop=(j == 7))
                    if j == 7:
                        nc.vector.tensor_scalar(
                            out=K_T[:, pr, nb * TB:(nb + 1) * TB],
                            in0=k_group.acc[:],
                            scalar1=kb_sb[:, pr:pr + 1],
                            scalar2=None, op0=ADD)
                return mm
            return [mk(j) for j in range(8)]

        def q_group(nb, pr, tag="qacc"):
            def mk(j):
                def mm():
                    if j == 0:
                        q_group.acc = ps.tile([128, TB], F32, tag=tag,
                                              bufs=1)
                    nc.tensor.matmul(
                        q_group.acc[:],
                        w_sb[:, j, pr * 128:(pr + 1) * 128],
                        x_tok(j, nb * TB, TB), start=(j == 0), stop=(j == 7))
                    if j == 7:
                        nc.vector.tensor_scalar(
                            out=Q_T[:, pr, nb * TB:(nb + 1) * TB],
                            in0=q_group.acc[:],
                            scalar1=qb_sb[:, pr:pr + 1],
                            scalar2=None, op0=ADD)
                return mm
            return [mk(j) for j in range(8)]

        def v_group(t, tag="qacc"):
            def mk(j):
                def mm():
                    if j == 0:
                        v_group.acc = ps.tile([128, TB], F32, tag=tag,
                                              bufs=1)
                    nc.tensor.matmul(v_group.acc[:],
                                     x_tok(j, t * 128, 128),
                                     w_sb[:, j, 1024:1536],
                                     start=(j == 0), stop=(j == 7))
                    if j == 7:
                        nc.vector.tensor_copy(
                            out=V_sb[:, t, :, 0:64],
                            in_=v_group.acc[:].rearrange(
                                "p (h d) -> p h d", h=8))
                return mm
            return [mk(j) for j in range(8)]

        # ---- prefix: K(pr0, nb0/nb1), V(t0-7), Q(qb0, pr0); the rest of
        # K(pr0) and V trickle into the PV-lag window.  Alternating psum
        # banks (pj is idle here) so each group's DVE evac overlaps the
        # next group's matmuls.
        tags = ["qacc", "pj"]
        gi = 0
        for nb in range(2):
            for f in k_group(nb, 0, tags[gi % 2]):
                f()
            gi += 1
        for f in q_group(0, 0, tags[gi % 2]):
            f()
        gi += 1

        # ---- trickle queue, pr-outer deadlines (first read of K(:,pr) is
        # step 32*pr; of Q(qb,pr) is step 32*pr + 8*qb), added in deadline
        # order.
        trickle = Trickle()
        pending = []
        for nb in (2, 3):
            pending.append((2 * nb - 2, k_group(nb, 0)))
        for t in range(8):
            pending.append((6 + (3 * t) // 8, v_group(t, tags[(gi + t) % 2])))
        for t in range(8, 16):
            pending.append((t // 2 + 6, v_group(t, tags[(gi + t) % 2])))
        for pr in range(1, 4):
            for nb in range(4):
                pending.append((32 * pr + 2 * nb - 6, k_group(nb, pr)))
        for pr in range(4):
            for qb_i in range(4):
                if qb_i == 0 and pr == 0:
                    continue
                pending.append((32 * pr + 8 * qb_i - 6, q_group(qb_i, pr)))
        for dl, grp in sorted(pending, key=lambda p: p[0]):
            trickle.add_group(dl, grp)

        proj_blocks = []

        def make_proj_block(O_qb, qb_i, ns, co, tag="pj", dq=None):
            def emit():
                pj = ps.tile([128, 512], F32, tag=tag, bufs=1)
                for pr in range(4):
                    nc.tensor.matmul(pj[:],
                                     O_qb[:, pr, ns * 128:(ns + 1) * 128],
                                     pw_sb[:, pr, co * 512:(co + 1) * 512],
                                     start=(pr == 0), stop=(pr == 3))
                so = sb.tile([128, 512], F32, tag="so", bufs=3)
                nc.vector.tensor_copy(out=so[:], in_=pj[:])
                (dq or nc.sync).dma_start(
                    out[qb_i * 512 + ns * 128:qb_i * 512 + (ns + 1) * 128,
                        co * 512:(co + 1) * 512], so[:])
            return emit

        norm_muls = []

        def emit_normalize(qb_i, pr, O_qb, oaug0, oaug1):
            # stage oaug into SBUF (frees the PSUM bank fast); denom row 64
            # -> reciprocal -> DRAM-bounce broadcast (per-head DMA queues).
            # The final multiply is deferred so it never blocks the
            # strict-FIFO DVE queue waiting on the bounce DMA.
            last = (qb_i == 3 and pr == 3)
            for hh, oaug in ((0, oaug0), (1, oaug1)):
                dq = nc.scalar if last else (nc.sync if hh == 0
                                             else nc.gpsimd)
                row = qb_i * 8 + pr * 2 + hh
                ou = sb.tile([65, 512], F32, tag="ou", bufs=6)
                nc.vector.tensor_copy(out=ou[:], in_=oaug[:])
                (nc.scalar if last else nc.sync).dma_start(
                    scratch[row:row + 1, :], ou[64:65, :])
                d8 = sb.tile([64, 8], F32, tag="d8", bufs=4)
                dq.dma_start(
                    d8[:], scratch[row:row + 1, :].rearrange(
                        "a (p f) -> (a p) f", p=64))
                r8 = sb.tile([64, 8], F32, tag="r8", bufs=4)
                nc.vector.reciprocal(r8[:], d8[:])
                dq.dma_start(
                    scratch2[row:row + 1, :].rearrange(
                        "a (p f) -> (a p) f", p=64), r8[:])
                rb = sb.tile([64, 512], F32, tag="rb", bufs=4)
                dq.dma_start(
                    rb[:], scratch2[row:row + 1, :].to_broadcast((64, 512)))

                def mul(hh=hh, ou=ou, rb=rb):
                    nc.vector.tensor_tensor(
                        out=O_qb[hh * 64:(hh + 1) * 64, pr, :],
                        in0=ou[0:64, :], in1=rb[:], op=MULT)
                norm_muls.append((emit_normalize.si + 2, mul))

        # ---- steady state: 128 steps, PR-OUTER (pr, qb, tg)
        steps = [(pr, qb, tg) for pr in range(4) for qb in range(4)
                 for tg in range(8)]
        O_qbs = {}
        ctx_of = {}
        proj_gate = [0]
        pv_q = []

        def make_pv(P_A, P_B, t0, t1, qb_i, pr, tg):
            def pv():
                if tg == 0:
                    if pr == 0:
                        oq_new = sb.tile([128, 4, 512], BF16, tag="oq",
                                         bufs=4)
                        O_qbs[qb_i] = oq_new
                    oaug0_new = ps.tile([65, 512], F32, tag="oaug", bufs=2)
                    oaug1_new = ps.tile([65, 512], F32, tag="oaug", bufs=2)
                    ctx_of[(qb_i, pr)] = (oaug0_new, oaug1_new, O_qbs[qb_i])
                po0, po1, pO = ctx_of[(qb_i, pr)]
                st, sp = (tg == 0), (tg == 7)
                nc.tensor.matmul(po0[:], V_sb[:, t0, 2 * pr, :],
                                 P_A[:, 0:512], start=st, stop=False)
                nc.tensor.matmul(po1[:], V_sb[:, t0, 2 * pr + 1, :],
                                 P_A[:, 512:1024], start=st, stop=False)
                nc.tensor.matmul(po0[:], V_sb[:, t1, 2 * pr, :],
                                 P_B[:, 0:512], start=False, stop=sp)
                nc.tensor.matmul(po1[:], V_sb[:, t1, 2 * pr + 1, :],
                                 P_B[:, 512:1024], start=False, stop=sp)
                if sp:
                    emit_normalize(qb_i, pr, pO, po0, po1)
                    if pr == 3:
                        proj_gate[0] = emit_normalize.si + 3
                        if qb_i != 3:  # qb3 handled by the pipelined tail
                            for ns in range(4):
                                for co in range(2):
                                    proj_blocks.append((pO, qb_i, ns, co))
            return pv

        for si, (pr, qb_i, tg) in enumerate(steps):
            emit_normalize.si = si
            q0 = qb_i * 512
            t0, t1 = 2 * tg, 2 * tg + 1
            trickle.emit(si, 3 if si < 16 else 2)
            # scores: stageA = key tile t0 x [h0 | h1], stageB = t1; the
            # (0,0)/(64,0) emission pairs co-run on the PE array.
            stageA = ps.tile([128, 1024], F32, tag="stage", bufs=2)
            stageB = ps.tile([128, 1024], F32, tag="stage", bufs=2)
            nc.tensor.matmul(stageA[:, 0:512],
                             K_T[0:64, pr, t0 * 128:(t0 + 1) * 128],
                             Q_T[0:64, pr, q0:q0 + 512],
                             start=True, stop=True, tile_position=(0, 0))
            nc.tensor.matmul(stageA[:, 512:1024],
                             K_T[64:128, pr, t0 * 128:(t0 + 1) * 128],
                             Q_T[64:128, pr, q0:q0 + 512],
                             start=True, stop=True, tile_position=(64, 0))
            nc.tensor.matmul(stageB[:, 0:512],
                             K_T[0:64, pr, t1 * 128:(t1 + 1) * 128],
                             Q_T[0:64, pr, q0:q0 + 512],
                             start=True, stop=True, tile_position=(0, 0))
            nc.tensor.matmul(stageB[:, 512:1024],
                             K_T[64:128, pr, t1 * 128:(t1 + 1) * 128],
                             Q_T[64:128, pr, q0:q0 + 512],
                             start=True, stop=True, tile_position=(64, 0))
            # lagged PV pops: the first 8 steps run scores/exp only (their
            # PE slack absorbs the V/K trickle); the backlog catches up at
            # one extra pop every other step.
            if si >= 8 and pv_q:
                pv_q.pop(0)()
                if ((si % 3 == 0 and len(pv_q) > 2)
                        or (si >= 100 and len(pv_q) > 1)):
                    pv_q.pop(0)()
            npop = 0
            while norm_muls and norm_muls[0][0] <= si and npop < 2:
                norm_muls.pop(0)[1]()
                npop += 1
            if proj_blocks and si >= proj_gate[0]:
                make_proj_block(*proj_blocks.pop(0))()
            # exp for this step
            P_A = sb.tile([128, 1024], BF16, tag="p", bufs=22)
            P_B = sb.tile([128, 1024], BF16, tag="p", bufs=22)
            nc.scalar.activation(P_A[:], stageA[:], EXP, bias=0.0,
                                 scale=SCALE)
            nc.scalar.activation(P_B[:], stageB[:], EXP, bias=0.0,
                                 scale=SCALE)
            pv_q.append(make_pv(P_A, P_B, t0, t1, qb_i, pr, tg))
        # ---- drain: remaining PVs (incl. the final normalize via the last
        # pv closure), then the pipelined tail projections
        trickle.drain()
        while pv_q:
            pv_q.pop(0)()
        pqb = 3
        pO = O_qbs[3]

        # tail: leftover steady blocks, then qb3's as partial/final pairs
        # with pj/qacc bank alternation; pr0-2 partials run before the
        # final normalize lands, spaced dummies keep HAM warm through the
        # bounce latency.
        def tail_partial(O_qb, qb_i, ns, co, tag):
            pj = ps.tile([128, 512], F32, tag=tag, bufs=1)
            for pr4 in range(3):
                nc.tensor.matmul(pj[:],
                                 O_qb[:, pr4, ns * 128:(ns + 1) * 128],
                                 pw_sb[:, pr4, co * 512:(co + 1) * 512],
                                 start=(pr4 == 0), stop=False)

            def fin():
                nc.tensor.matmul(pj[:],
                                 O_qb[:, 3, ns * 128:(ns + 1) * 128],
                                 pw_sb[:, 3, co * 512:(co + 1) * 512],
                                 start=False, stop=True)
                so = sb.tile([128, 512], F32, tag="so", bufs=3)
                nc.vector.tensor_copy(out=so[:], in_=pj[:])
                dq = (nc.sync, nc.gpsimd, nc.scalar)[(ns * 2 + co) % 3]
                dq.dma_start(
                    out[qb_i * 512 + ns * 128:qb_i * 512 + (ns + 1) * 128,
                        co * 512:(co + 1) * 512], so[:])
            return fin

        # any steady-state leftovers first (no pr3 dependency issues: their
        # qb finished long ago); re-emit them with alternating psum banks
        # and DMA queues so evacs overlap the next block's matmuls
        nleft = len(proj_blocks)
        for bi, spec in enumerate(proj_blocks):
            make_proj_block(*spec, tag="pj" if bi % 2 == 0 else "qacc",
                            dq=(nc.sync, nc.gpsimd, nc.scalar)[bi % 3])()
        proj_blocks = []
        # qb3 blocks as partial/final pairs
        fins = []
        specs = [(ns, co) for ns in range(4) for co in range(2)]
        for bi, (ns, co) in enumerate(specs[:2]):
            fins.append(tail_partial(pO, pqb, ns, co,
                                     "pj" if bi % 2 == 0 else "qacc"))
        # keep-warm dummies during the normalize bounce round-trip
        for _ in range(4):
            dummy = ps.tile([128, 512], F32, tag="stage", bufs=2)
            nc.tensor.matmul(dummy[:], K_T[0:64, 0, 0:128],
                             Q_T[0:64, 0, 0:512], start=True, stop=True)
        while norm_muls:
            norm_muls.pop(0)[1]()
        for bi in range(len(specs)):
            fins.pop(0)()
            nxt = bi + 2
            if nxt < len(specs):
                ns, co = specs[nxt]
                fins.append(tail_partial(pO, pqb, ns, co,
                                         "pj" if nxt % 2 == 0 else "qacc"))
    return nc


def _prepare_in_maps(x, qkv_w, qkv_b, proj_w):
    x = np.asarray(x, dtype=np.float32)
    w = np.asarray(qkv_w, dtype=np.float32)
    pwr = np.asarray(proj_w, dtype=np.float32)
    qkv_b = np.asarray(qkv_b, dtype=np.float32)
    in_maps = []
    for c in range(8):
        b, g = c % 4, c // 4
        w0 = 512 * g
        xt = np.ascontiguousarray(x[b].T)
        in_maps.append({
            "xT16": _bf16(xt),
            "wcat": _bf16(np.concatenate(
                [w[:, w0:w0 + 512],
                 w[:, 1024 + w0:1024 + w0 + 512],
                 w[:, 2048 + w0:2048 + w0 + 512]], axis=1)),
            "qb": np.ascontiguousarray(qkv_b[w0:w0 + 512].reshape(4, 128).T),
            "kb": np.ascontiguousarray(
                qkv_b[1024 + w0:1024 + w0 + 512].reshape(4, 128).T),
            "pw": _bf16(pwr[w0:w0 + 512, :]),
        })
    return in_maps


def _gather(parts, qkv_b, proj_w, proj_b):
    const_row = (np.asarray(qkv_b)[2048:].astype(np.float64)
                 @ np.asarray(proj_w).astype(np.float64)
                 + np.asarray(proj_b).astype(np.float64))
    out = np.empty((B, N, C), np.float32)
    for b in range(B):
        out[b] = (parts[b].astype(np.float64) + parts[b + 4].astype(np.float64)
                  + const_row).astype(np.float32)
    return out


def kernel(**inputs: np.ndarray) -> np.ndarray:
    x = np.asarray(inputs["x"], dtype=np.float32)
    qkv_w = np.asarray(inputs["qkv_w"], dtype=np.float32)
    qkv_b = np.asarray(inputs["qkv_b"], dtype=np.float32)
    proj_w = np.asarray(inputs["proj_w"], dtype=np.float32)
    proj_b = np.asarray(inputs["proj_b"], dtype=np.float32)

    in_maps = _prepare_in_maps(x, qkv_w, qkv_b, proj_w)
    nc = _build()
    nc.finalize()
    res = run_bass_kernel_spmd(nc, in_maps, list(range(8)))
    parts = [res.results[c]["out"] for c in range(8)]
    return _gather(parts, qkv_b, proj_w, proj_b)


if __name__ == "__main__":
    import tempfile
    import time

    from concourse.bass_utils import compile_bass_kernel

    t0 = time.time()
    nc = _build()
    nc.compile()
    with tempfile.TemporaryDirectory() as td:
        compile_bass_kernel(nc, td, neff_name="k.neff")
    print(f"COMPILE OK ({time.time() - t0:.0f}s)", flush=True)


# revision 23
# speedup vs baseline: 1.0318x; 1.0318x over previous
"""Multi-head attention (B=4, N=2048, C=1024, H=16, D=64) on 8 TRN2 cores.

Sharding: core c -> batch b = c%4, head-group g = c//4 (local heads 0..7 are
global heads 8g..8g+7).  Each core computes its head group's contribution to
the output projection for its batch; host sums core b + core b+4 and adds
const_row = qkv_b[2048:] @ proj_w + proj_b (V-bias folds exactly through the
row-normalized attention: attn @ (1*bv^T) = 1*bv^T).

All matmul operands bf16 (fp32 PSUM accumulation).  Structure (v3):
- Input DMAs: x as 8 full 4KB-line rows spread over 4 queues; w full rows.
- Prefix: V (all tokens), K (pr0), Q (qb0,pr0) -- ~36us, DMA-overlapped.
- Steady state: 128 steps, PR-OUTER order (pr, qb, tg) so the remaining
  K (pr1-3) / Q projection groups trickle at a uniform ~1.7 matmuls/step
  into per-step PE slack (deadline-driven queue; qacc bank bufs=1).
  ACT-paced at 2x1147ns/step.
- Score PSUM staging grouped BY KEY TILE (stageA = t0 x [h0|h1]) so the
  tile_position row-half pair (0,0)/(64,0) shares one exp dependency,
  stays adjacent in the PE queue, and co-runs on the PE array.
- PSUM budget (8 banks): stage 2x2 + oaug 2x1 + qacc 1 + pj 1.
- Softmax denominator rides as a 65th ones-column in V (attn@1 = rowsum).
  Normalization: denom row -> SBUF -> DRAM bounce reshaped [64,8] for a
  wide DVE reciprocal -> broadcast read -> deferred multiply; the two
  heads' bounce chains use different DMA queues.
- Projections: with pr-outer order O_qb(qb) completes only after its pr3
  normalize, so proj blocks run 1/step over the last ~24 steps and qb3's
  in a pipelined tail (pj/qacc bank alternation, pr0-2 partials emitted
  early, a few spaced dummies keep HAM warm through the bounce latency).
- gpsimd partition_broadcast / reciprocal_approx_fast are numerically
  broken on this hardware - do not reintroduce them.
"""

import sys

sys.path.insert(0, "/opt/trn_rl_repo")

from contextlib import ExitStack

import ml_dtypes
import numpy as np

from concourse import bacc, mybir, tile
from concourse.bass_utils import run_bass_kernel_spmd

F32 = mybir.dt.float32
BF16 = mybir.dt.bfloat16
EXP = mybir.ActivationFunctionType.Exp
ADD = mybir.AluOpType.add
MULT = mybir.AluOpType.mult

B, N, C, H, D = 4, 2048, 1024, 16, 64
SCALE = 0.125
TB = 512


def _bf16(a: np.ndarray) -> np.ndarray:
    return np.ascontiguousarray(a, dtype=np.float32).astype(ml_dtypes.bfloat16)


class Trickle:
    """Deadline-driven queue of single-matmul closures fed into PE slack."""

    def __init__(self):
        self.items = []

    def add_group(self, deadline, closures):
        for c in closures:
            self.items.append((deadline, c))

    def emit(self, step, quota):
        n = 0
        while self.items and (self.items[0][0] <= step + 1 or n < quota):
            self.items.pop(0)[1]()
            n += 1

    def drain(self):
        while self.items:
            self.items.pop(0)[1]()


def _build():
    nc = bacc.Bacc("TRN2", target_bir_lowering=False, debug=False)
    xT16 = nc.dram_tensor("xT16", [1024, 2048], BF16, kind="ExternalInput").ap()
    wcat = nc.dram_tensor("wcat", [1024, 1536], BF16, kind="ExternalInput").ap()
    qb = nc.dram_tensor("qb", [128, 4], F32, kind="ExternalInput").ap()
    kb = nc.dram_tensor("kb", [128, 4], F32, kind="ExternalInput").ap()
    pw = nc.dram_tensor("pw", [512, 1024], BF16, kind="ExternalInput").ap()
    out = nc.dram_tensor("out", [2048, 1024], F32, kind="ExternalOutput").ap()
    scratch = nc.dram_tensor("scratch", [32, 512], F32).ap()
    scratch2 = nc.dram_tensor("scratch2", [32, 512], F32).ap()

    with tile.TileContext(nc) as tc, ExitStack() as ctx:
        sb = ctx.enter_context(tc.tile_pool(name="sb", bufs=1))
        ps = ctx.enter_context(tc.tile_pool(name="ps", bufs=1, space="PSUM"))

        w_sb = sb.tile([128, 8, 1536], BF16, tag="w")
        pw_sb = sb.tile([128, 4, 1024], BF16, tag="pw")
        Q_T = sb.tile([128, 4, 2048], BF16, tag="qt")
        K_T = sb.tile([128, 4, 2048], BF16, tag="kt")
        V_sb = sb.tile([128, 16, 8, 65], BF16, tag="v")
        x_sb = sb.tile([128, 8, 2048], BF16, tag="x")
        qb_sb = sb.tile([128, 4], F32, tag="qb")
        kb_sb = sb.tile([128, 4], F32, tag="kb")
        zc = sb.tile([128, 8, 1], F32, tag="zc")
        onec = sb.tile([128, 1], F32, tag="onec")
        warm = sb.tile([128, 4], F32, tag="warm")

        # ---- initial DMAs over the three DMA-capable queues, ordered by
        # first use: x token-halves in two waves (the prefix only needs
        # tokens 0-1023 early), w rows spread 4/2/2 so the V columns land
        # by ~12us.
        # prefix-critical 6MB (x + w KV-columns) balanced 2MB/engine and
        # interleaved so K/V chunk compute pipelines with arrivals; the w
        # Q-columns, second x wave, biases and pw follow.
        for j in range(0, 8, 2):
            nc.sync.dma_start(x_sb[:, j, 0:1024],
                              xT16[j * 128:(j + 1) * 128, 0:1024])
            nc.gpsimd.dma_start(x_sb[:, j + 1, 0:1024],
                                xT16[(j + 1) * 128:(j + 2) * 128, 0:1024])
            if j < 8:
                nc.scalar.dma_start(w_sb[:, j // 2, 512:1536],
                                    wcat[(j // 2) * 128:
                                         (j // 2 + 1) * 128, 512:1536])
        nc.sync.dma_start(w_sb[:, 4, 512:1536], wcat[4 * 128:5 * 128,
                                                     512:1536])
        nc.sync.dma_start(w_sb[:, 6, 512:1536], wcat[6 * 128:7 * 128,
                                                     512:1536])
        nc.gpsimd.dma_start(w_sb[:, 5, 512:1536], wcat[5 * 128:6 * 128,
                                                       512:1536])
        nc.gpsimd.dma_start(w_sb[:, 7, 512:1536], wcat[7 * 128:8 * 128,
                                                       512:1536])
        nc.scalar.dma_start(kb_sb[:], kb[:])
        nc.scalar.dma_start(qb_sb[:], qb[:])
        for j in range(4):  # Q-columns, first needed ~step 0-16
            nc.sync.dma_start(w_sb[:, j, 0:512],
                              wcat[j * 128:(j + 1) * 128, 0:512])
            nc.gpsimd.dma_start(w_sb[:, j + 4, 0:512],
                                wcat[(j + 4) * 128:(j + 5) * 128, 0:512])
        for j in range(0, 8, 2):
            nc.sync.dma_start(x_sb[:, j, 1024:2048],
                              xT16[j * 128:(j + 1) * 128, 1024:2048])
            nc.gpsimd.dma_start(x_sb[:, j + 1, 1024:2048],
                                xT16[(j + 1) * 128:(j + 2) * 128, 1024:2048])
        for pr in range(4):
            nc.scalar.dma_start(pw_sb[:, pr, :],
                                pw[pr * 128:(pr + 1) * 128, :])

        def x_tok(j, lo, n):
            return x_sb[:, j, lo:lo + n]

        nc.vector.memset(zc[:], 0.0)
        nc.vector.memset(onec[:], 1.0)
        # preload the exp table set while the prefix runs
        nc.scalar.activation(warm[0:1, 0:1], onec[0:1, 0:1], EXP,
                             bias=0.0, scale=1.0)
        for t in range(16):
            nc.vector.tensor_scalar(out=V_sb[:, t, :, 64:65], in0=zc[:],
                                    scalar1=onec[:], scalar2=None, op0=ADD)

        # ---- projection-group emitters (shared by prefix and trickle)
        def k_group(nb, pr, tag="qacc"):
            def mk(j):
                def mm():
                    if j == 0:
                        k_group.acc = ps.tile([128, TB], F32, tag=tag,
                                              bufs=1)
                    nc.tensor.matmul(
                        k_group.acc[:],
                        w_sb[:, j, 512 + pr * 128:512 + (pr + 1) * 128],
                        x_tok(j, nb * TB, TB), start=(j == 0), stop=(j == 7))
                    if j == 7:
                        nc.vector.tensor_scalar(
                            out=K_T[:, pr, nb * TB:(nb + 1) * TB],
                            in0=k_group.acc[:],
                            scalar1=kb_sb[:, pr:pr + 1],
                            scalar2=None, op0=ADD)
                return mm
            return [mk(j) for j in range(8)]

        def q_group(nb, pr, tag="qacc"):
            def mk(j):
                def mm():
                    if j == 0:
                        q_group.acc = ps.tile([128, TB], F32, tag=tag,
                                              bufs=1)
                    nc.tensor.matmul(
                        q_group.acc[:],
                        w_sb[:, j, pr * 128:(pr + 1) * 128],
                        x_tok(j, nb * TB, TB), start=(j == 0), stop=(j == 7))
                    if j == 7:
                        nc.vector.tensor_scalar(
                            out=Q_T[:, pr, nb * TB:(nb + 1) * TB],
                            in0=q_group.acc[:],
                            scalar1=qb_sb[:, pr:pr + 1],
                            scalar2=None, op0=ADD)
                return mm
            return [mk(j) for j in range(8)]

        def v_group(t, tag="qacc"):
            def mk(j):
                def mm():
                    if j == 0:
                        v_group.acc = ps.tile([128, TB], F32, tag=tag,
                                              bufs=1)
                    nc.tensor.matmul(v_group.acc[:],
                                     x_tok(j, t * 128, 128),
                                     w_sb[:, j, 1024:1536],
                                     start=(j == 0), stop=(j == 7))
                    if j == 7:
                        nc.vector.tensor_copy(
                            out=V_sb[:, t, :, 0:64],
                            in_=v_group.acc[:].rearrange(
                                "p (h d) -> p h d", h=8))
                return mm
            return [mk(j) for j in range(8)]

        # ---- prefix: K(pr0, nb0/nb1), V(t0-7), Q(qb0, pr0); the rest of
        # K(pr0) and V trickle into the PV-lag window.  Alternating psum
        # banks (pj is idle here) so each group's DVE evac overlaps the
        # next group's matmuls.
        tags = ["qacc", "pj"]
        gi = 0
        for nb in range(2):
            for f in k_group(nb, 0, tags[gi % 2]):
                f()
            gi += 1
        for t in range(4):
            for f in v_group(t, tags[gi % 2]):
                f()
            gi += 1
        for f in q_group(0, 0, tags[gi % 2]):
            f()
        gi += 1
        for t in range(4, 8):
            for f in v_group(t, tags[gi % 2]):
                f()
            gi += 1

        # ---- trickle queue, pr-outer deadlines (first read of K(:,pr) is
        # step 32*pr; of Q(qb,pr) is step 32*pr + 8*qb), added in deadline
        # order.
        trickle = Trickle()
        pending = []
        for nb in (2, 3):
            pending.append((2 * nb - 2, k_group(nb, 0)))
        for t in range(8, 16):
            pending.append((t // 2 + 6, v_group(t, tags[(gi + t) % 2])))
        for pr in range(1, 4):
            for nb in range(4):
                pending.append((32 * pr + 2 * nb - 6, k_group(nb, pr)))
        for pr in range(4):
            for qb_i in range(4):
                if qb_i == 0 and pr == 0:
                    continue
                pending.append((32 * pr + 8 * qb_i - 6, q_group(qb_i, pr)))
        for dl, grp in sorted(pending, key=lambda p: p[0]):
            trickle.add_group(dl, grp)

        proj_blocks = []

        def make_proj_block(O_qb, qb_i, ns, co, tag="pj", dq=None):
            def emit():
                pj = ps.tile([128, 512], F32, tag=tag, bufs=1)
                for pr in range(4):
                    nc.tensor.matmul(pj[:],
                                     O_qb[:, pr, ns * 128:(ns + 1) * 128],
                                     pw_sb[:, pr, co * 512:(co + 1) * 512],
                                     start=(pr == 0), stop=(pr == 3))
                so = sb.tile([128, 512], F32, tag="so", bufs=3)
                nc.vector.tensor_copy(out=so[:], in_=pj[:])
                (dq or nc.sync).dma_start(
                    out[qb_i * 512 + ns * 128:qb_i * 512 + (ns + 1) * 128,
                        co * 512:(co + 1) * 512], so[:])
            return emit

        norm_muls = []

        def emit_normalize(qb_i, pr, O_qb, oaug0, oaug1):
            # stage oaug into SBUF (frees the PSUM bank fast); denom row 64
            # -> reciprocal -> DRAM-bounce broadcast (per-head DMA queues).
            # The final multiply is deferred so it never blocks the
            # strict-FIFO DVE queue waiting on the bounce DMA.
            last = (qb_i == 3 and pr == 3)
            for hh, oaug in ((0, oaug0), (1, oaug1)):
                dq = nc.scalar if last else (nc.sync if hh == 0
                                             else nc.gpsimd)
                row = qb_i * 8 + pr * 2 + hh
                ou = sb.tile([65, 512], F32, tag="ou", bufs=6)
                nc.vector.tensor_copy(out=ou[:], in_=oaug[:])
                (nc.scalar if last else nc.sync).dma_start(
                    scratch[row:row + 1, :], ou[64:65, :])
                d8 = sb.tile([64, 8], F32, tag="d8", bufs=4)
                dq.dma_start(
                    d8[:], scratch[row:row + 1, :].rearrange(
                        "a (p f) -> (a p) f", p=64))
                r8 = sb.tile([64, 8], F32, tag="r8", bufs=4)
                nc.vector.reciprocal(r8[:], d8[:])
                dq.dma_start(
                    scratch2[row:row + 1, :].rearrange(
                        "a (p f) -> (a p) f", p=64), r8[:])
                rb = sb.tile([64, 512], F32, tag="rb", bufs=4)
                dq.dma_start(
                    rb[:], scratch2[row:row + 1, :].to_broadcast((64, 512)))

                def mul(hh=hh, ou=ou, rb=rb):
                    nc.vector.tensor_tensor(
                        out=O_qb[hh * 64:(hh + 1) * 64, pr, :],
                        in0=ou[0:64, :], in1=rb[:], op=MULT)
                norm_muls.append((emit_normalize.si + 2, mul))

        # ---- steady state: 128 steps, PR-OUTER (pr, qb, tg)
        steps = [(pr, qb, tg) for pr in range(4) for qb in range(4)
                 for tg in range(8)]
        O_qbs = {}
        ctx_of = {}
        proj_gate = [0]
        pv_q = []

        def make_pv(P_A, P_B, t0, t1, qb_i, pr, tg):
            def pv():
                if tg == 0:
                    if pr == 0:
                        oq_new = sb.tile([128, 4, 512], BF16, tag="oq",
                                         bufs=4)
                        O_qbs[qb_i] = oq_new
                    oaug0_new = ps.tile([65, 512], F32, tag="oaug", bufs=2)
                    oaug1_new = ps.tile([65, 512], F32, tag="oaug", bufs=2)
                    ctx_of[(qb_i, pr)] = (oaug0_new, oaug1_new, O_qbs[qb_i])
                po0, po1, pO = ctx_of[(qb_i, pr)]
                st, sp = (tg == 0), (tg == 7)
                nc.tensor.matmul(po0[:], V_sb[:, t0, 2 * pr, :],
                                 P_A[:, 0:512], start=st, stop=False)
                nc.tensor.matmul(po1[:], V_sb[:, t0, 2 * pr + 1, :],
                                 P_A[:, 512:1024], start=st, stop=False)
                nc.tensor.matmul(po0[:], V_sb[:, t1, 2 * pr, :],
                                 P_B[:, 0:512], start=False, stop=sp)
                nc.tensor.matmul(po1[:], V_sb[:, t1, 2 * pr + 1, :],
                                 P_B[:, 512:1024], start=False, stop=sp)
                if sp:
                    emit_normalize(qb_i, pr, pO, po0, po1)
                    if pr == 3:
                        proj_gate[0] = emit_normalize.si + 3
                        if qb_i != 3:  # qb3 handled by the pipelined tail
                            for ns in range(4):
                                for co in range(2):
                                    proj_blocks.append((pO, qb_i, ns, co))
            return pv

        for si, (pr, qb_i, tg) in enumerate(steps):
            emit_normalize.si = si
            q0 = qb_i * 512
            t0, t1 = 2 * tg, 2 * tg + 1
            trickle.emit(si, 3 if si < 16 else 2)
            # scores: stageA = key tile t0 x [h0 | h1], stageB = t1; the
            # (0,0)/(64,0) emission pairs co-run on the PE array.
            stageA = ps.tile([128, 1024], F32, tag="stage", bufs=2)
            stageB = ps.tile([128, 1024], F32, tag="stage", bufs=2)
            nc.tensor.matmul(stageA[:, 0:512],
                             K_T[0:64, pr, t0 * 128:(t0 + 1) * 128],
                             Q_T[0:64, pr, q0:q0 + 512],
                             start=True, stop=True, tile_position=(0, 0))
            nc.tensor.matmul(stageA[:, 512:1024],
                             K_T[64:128, pr, t0 * 128:(t0 + 1) * 128],
                             Q_T[64:128, pr, q0:q0 + 512],
                             start=True, stop=True, tile_position=(64, 0))
            nc.tensor.matmul(stageB[:, 0:512],
                             K_T[0:64, pr, t1 * 128:(t1 + 1) * 128],
                             Q_T[0:64, pr, q0:q0 + 512],
                             start=True, stop=True, tile_position=(0, 0))
            nc.tensor.matmul(stageB[:, 512:1024],
                             K_T[64:128, pr, t1 * 128:(t1 + 1) * 128],
                             Q_T[64:128, pr, q0:q0 + 512],
                             start=True, stop=True, tile_position=(64, 0))
            # lagged PV pops: the first 8 steps run scores/exp only (their
            # PE slack absorbs the V/K trickle); the backlog catches up at
            # one extra pop every other step.
            if si >= 8 and pv_q:
                pv_q.pop(0)()
                if ((si % 3 == 0 and len(pv_q) > 2)
                        or (si >= 100 and len(pv_q) > 1)):
                    pv_q.pop(0)()
            npop = 0
            while norm_muls and norm_muls[0][0] <= si and npop < 2:
                norm_muls.pop(0)[1]()
                npop += 1
            if proj_blocks and si >= proj_gate[0]:
                make_proj_block(*proj_blocks.pop(0))()
            # exp for this step
            P_A = sb.tile([128, 1024], BF16, tag="p", bufs=22)
            P_B = sb.tile([128, 1024], BF16, tag="p", bufs=22)
            nc.scalar.activation(P_A[:], stageA[:], EXP, bias=0.0,
                                 scale=SCALE)
            nc.scalar.activation(P_B[:], stageB[:], EXP, bias=0.0,
                                 scale=SCALE)
            pv_q.append(make_pv(P_A, P_B, t0, t1, qb_i, pr, tg))
        # ---- drain: remaining PVs (incl. the final normalize via the last
        # pv closure), then the pipelined tail projections
        trickle.drain()
        while pv_q:
            pv_q.pop(0)()
        pqb = 3
        pO = O_qbs[3]

        # tail: leftover steady blocks, then qb3's as partial/final pairs
        # with pj/qacc bank alternation; pr0-2 partials run before the
        # final normalize lands, spaced dummies keep HAM warm through the
        # bounce latency.
        def tail_partial(O_qb, qb_i, ns, co, tag):
            pj = ps.tile([128, 512], F32, tag=tag, bufs=1)
            for pr4 in range(3):
                nc.tensor.matmul(pj[:],
                                 O_qb[:, pr4, ns * 128:(ns + 1) * 128],
                                 pw_sb[:, pr4, co * 512:(co + 1) * 512],
                                 start=(pr4 == 0), stop=False)

            def fin():
                nc.tensor.matmul(pj[:],
                                 O_qb[:, 3, ns * 128:(ns + 1) * 128],
                                 pw_sb[:, 3, co * 512:(co + 1) * 512],
                                 start=False, stop=True)
                so = sb.tile([128, 512], F32, tag="so", bufs=3)
                nc.vector.tensor_copy(out=so[:], in_=pj[:])
                dq = (nc.sync, nc.gpsimd, nc.scalar)[(ns * 2 + co) % 3]
                dq.dma_start(
                    out[qb_i * 512 + ns * 128:qb_i * 512 + (ns + 1) * 128,
                        co * 512:(co + 1) * 512], so[:])
            return fin

        # any steady-state leftovers first (no pr3 dependency issues: their
        # qb finished long ago); re-emit them with alternating psum banks
        # and DMA queues so evacs overlap the next block's matmuls
        nleft = len(proj_blocks)
        for bi, spec in enumerate(proj_blocks):
            make_proj_block(*spec, tag="pj" if bi % 2 == 0 else "qacc",
                            dq=(nc.sync, nc.gpsimd, nc.scalar)[bi % 3])()
        proj_blocks = []
        # qb3 blocks as partial/final pairs
        fins = []
        specs = [(ns, co) for ns in range(4) for co in range(2)]
        for bi, (ns, co) in enumerate(specs[:2]):
            fins.append(tail_partial(pO, pqb, ns, co,
                                     "pj" if bi % 2 == 0 else "qacc"))
        # keep-warm dummies during the normalize bounce round-trip
        for _ in range(4):
            dummy = ps.tile([128, 512], F32, tag="stage", bufs=2)
            nc.tensor.matmul(dummy[:], K_T[0:64, 0, 0:128],
                             Q_T[0:64, 0, 0:512], start=True, stop=True)
        while norm_muls:
            norm_muls.pop(0)[1]()
        for bi in range(len(specs)):
            fins.pop(0)()
            nxt = bi + 2
            if nxt < len(specs):
                ns, co = specs[nxt]
                fins.append(tail_partial(pO, pqb, ns, co,
                                         "pj" if nxt % 2 == 0 else "qacc"))
    return nc


def _prepare_in_maps(x, qkv_w, qkv_b, proj_w):
    x = np.asarray(x, dtype=np.float32)
    w = np.asarray(qkv_w, dtype=np.float32)
    pwr = np.asarray(proj_w, dtype=np.float32)
    qkv_b = np.asarray(qkv_b, dtype=np.float32)
    in_maps = []
    for c in range(8):
        b, g = c % 4, c // 4
        w0 = 512 * g
        xt = np.ascontiguousarray(x[b].T)
        in_maps.append({
            "xT16": _bf16(xt),
            "wcat": _bf16(np.concatenate(
                [w[:, w0:w0 + 512],
                 w[:, 1024 + w0:1024 + w0 + 512],
                 w[:, 2048 + w0:2048 + w0 + 512]], axis=1)),
            "qb": np.ascontiguousarray(qkv_b[w0:w0 + 512].reshape(4, 128).T),
            "kb": np.ascontiguousarray(
                qkv_b[1024 + w0:1024 + w0 + 512].reshape(4, 128).T),
            "pw": _bf16(pwr[w0:w0 + 512, :]),
        })
    return in_maps


def _gather(parts, qkv_b, proj_w, proj_b):
    const_row = (np.asarray(qkv_b)[2048:].astype(np.float64)
                 @ np.asarray(proj_w).astype(np.float64)
                 + np.asarray(proj_b).astype(np.float64))
    out = np.empty((B, N, C), np.float32)
    for b in range(B):
        out[b] = (parts[b].astype(np.float64) + parts[b + 4].astype(np.float64)
                  + const_row).astype(np.float32)
    return out


def kernel(**inputs: np.ndarray) -> np.ndarray:
    x = np.asarray(inputs["x"], dtype=np.float32)
    qkv_w = np.asarray(inputs["qkv_w"], dtype=np.float32)
    qkv_b = np.asarray(inputs["qkv_b"], dtype=np.float32)
    proj_w = np.asarray(inputs["proj_w"], dtype=np.float32)
    proj_b = np.asarray(inputs["proj_b"], dtype=np.float32)

    in_maps = _prepare_in_maps(x, qkv_w, qkv_b, proj_w)
    nc = _build()
    nc.finalize()
    res = run_bass_kernel_spmd(nc, in_maps, list(range(8)))
    parts = [res.results[c]["out"] for c in range(8)]
    return _gather(parts, qkv_b, proj_w, proj_b)


if __name__ == "__main__":
    import tempfile
    import time

    from concourse.bass_utils import compile_bass_kernel

    t0 = time.time()
    nc = _build()
    nc.compile()
    with tempfile.TemporaryDirectory() as td:
        compile_bass_kernel(nc, td, neff_name="k.neff")
    print(f"COMPILE OK ({time.time() - t0:.0f}s)", flush=True)
